# revision 17
# baseline (speedup 1.0000x reference)
"""ANI-style element-MLP (MoE routing) kernel for 8 TRN2 NeuronCores.

Strategy (v2):
  - Host: bucket atoms by element (expert); cores 2e, 2e+1 own expert e,
    928 slots each (capacity 1856/expert covers the ~1850 max count).
    Per-core inputs are packed into two byte-blobs so the whole input
    lands in 6 logical DMAs (3 per HWDGE ring, partition-split 0:64 /
    64:128 so the two rings use disjoint SDMA engines):
      f8 blob  (fp8 e4m3): W1 tiles | x chunk0 | x chunk1
      wb blob  (bf16)    : W2 tiles | W3 cols | b1 / b3 bias cols
      brow     (bf16)    : [1,512] b1|b2_eff row for rank-1 bias matmuls
  - Device: L1 runs in fp8 with DoubleRow (K=256 per pass: d0|d1 blocked
    halves), d2 as a plain fp8 K=128 pass.  L2/L3 in bf16.  Softplus =
    EXP then LN(1+t) on the ACT engine; chunk-0 L1 EXP is h-split with
    the per-partition ACT bias (saves the cold-PE bias matmuls on the
    critical path), all other biases ride rank-1 ones-row matmuls.
    The -log(2) shift is folded into downstream biases on host.
  - PE warmup matmuls run before the x DMA lands to keep the HAM clock
    monitor fed (2.4 GHz boost).
  - Host: scatter-add real slots' energies into the per-molecule output.

Self-contained: hardcodes problem shapes B=32, N=512, D=384, E=4, H=256.
"""

import ml_dtypes
import numpy as np

import concourse.bass as bass  # noqa: F401  (bass types referenced via bacc/mybir)
import concourse.mybir as mybir
from concourse import bacc
from concourse.bass_utils import run_bass_kernel_spmd
from concourse.hw_specs import get_activation_tables

class _OneActSetBacc(bacc.Bacc):
    """All our ACT functions (Exp, Ln, Identity) live in the
    natural_log_exp_and_others table set, but the stock table-load pass
    assigns each function its first matching set, thrashing ~1.5us table
    loads between sets on every layer.  Force every load to the one set
    that covers all three and drop the now-redundant reloads."""

    _ACT_SET = "natural_log_exp_and_others"

    def insert_act_table_loads(self):
        super().insert_act_table_loads()
        names = list(get_activation_tables(self.m.arch))
        target = names.index(self._ACT_SET)
        for blk in self.main_func.blocks:
            seen_engines = set()
            to_remove = []
            for inst in blk.instructions:
                if isinstance(inst, mybir.InstLoadActFuncSet):
                    if inst.engine in seen_engines and not (inst.has_wait() or inst.has_update()):
                        to_remove.append(inst)
                    else:
                        inst.act_func_set_id = target
                        seen_engines.add(inst.engine)
            for inst in to_remove:
                blk.instructions.remove(inst)

LOG2 = np.float32(np.log(2.0))
B, N, D = 32, 512, 384
E = 4
H1 = H2 = 256
N_CORES = 8
NT = 512   # legacy fallback tile
SC = 480   # v2 slots per chunk (2 chunks per core); capacity 4*SC per expert
SV = 2 * SC  # 960 slots per core
XC = 3 * SC  # x columns per chunk (d0|d1|d2 tiles)
F8C = 768 + 2 * XC  # f8 blob columns: wtA | x chunk0 | x chunk1

F32 = mybir.dt.float32
BF16 = mybir.dt.bfloat16
F8 = mybir.dt.float8e4

# Set by test harnesses: PROFILE=True makes kernel() run with NTFF tracing and
# store the profiled NEFF exec time (ns) in LAST_EXEC_NS.
PROFILE = False
TRACE_CORES = [0]
LAST_EXEC_NS = None

_CACHE: dict = {}

N_WARM = 7  # PE warmup matmuls issued while the x DMA streams in


def _build_v2():
    """Raw-Bass per-core graph, 928 slots (2 chunks of 464), fp8 L1.

    Engine plan (explicit semaphores, no Tile):
      sync   : ring A DMAs (partitions 0:64 of both blobs + brow), out t0
      scalar : ring B DMAs (partitions 64:128), all EXP/LN, out t1
      tensor : warmup + L1 (fp8 DoubleRow) + L2/L3 (bf16) + bias matmuls
      vector : warm/ones setup, +b3 epilogues PSUM->SBUF
    """
    from contextlib import ExitStack

    nc = _OneActSetBacc(None, target_bir_lowering=False)

    # f8 blob: wtA [128,768] | x chunk0 [128,XC] | x chunk1 [128,XC]
    f8_ext = nc.declare_dram_parameter("f8", [128, F8C], F8, isOutput=False)
    # wb blob: W2 tiles 512 | W3 2 | b1h0 b1h1 (ACT bias) 2
    wb_ext = nc.declare_dram_parameter("wb", [128, 516], BF16, isOutput=False)
    brow_ext = nc.declare_dram_parameter("brow", [1, 512], BF16, isOutput=False)
    bv_ext = nc.declare_dram_parameter("bv", [1, 8], F32, isOutput=False)
    out_ext = nc.declare_dram_parameter("out", [1, SV], F32, isOutput=True)

    EXP = mybir.ActivationFunctionType.Exp
    LN = mybir.ActivationFunctionType.Ln
    DR = mybir.MatmulPerfMode.DoubleRow

    with ExitStack() as ctx:
        f8sb = ctx.enter_context(nc.sbuf_tensor([128, F8C], F8))
        wb = ctx.enter_context(nc.sbuf_tensor([128, 516], BF16))
        bv = ctx.enter_context(nc.sbuf_tensor([1, 8], F32))
        brow_sb = ctx.enter_context(nc.sbuf_tensor([1, 512], BF16))
        warm = ctx.enter_context(nc.sbuf_tensor([128, 512], BF16))
        ones = ctx.enter_context(nc.sbuf_tensor([1, SC], BF16))
        scratch = ctx.enter_context(nc.sbuf_tensor([1, 16], F32))
        out_sb = ctx.enter_context(nc.sbuf_tensor([1, SV], F32))
        t1f = [ctx.enter_context(nc.sbuf_tensor(f"t1f_{t}", [128, 1024], F32)) for t in range(2)]
        a1 = [ctx.enter_context(nc.sbuf_tensor(f"a1_{t}", [128, 1024], BF16)) for t in range(2)]
        t2f = [ctx.enter_context(nc.sbuf_tensor(f"t2f_{t}", [128, 1024], F32)) for t in range(2)]
        a2 = [ctx.enter_context(nc.sbuf_tensor(f"a2_{t}", [128, 1024], BF16)) for t in range(2)]
        z1 = [ctx.enter_context(nc.psum_tensor(f"z1_{t}", [128, 1024], F32)) for t in range(2)]
        z2 = [ctx.enter_context(nc.psum_tensor(f"z2_{t}", [128, 1024], F32)) for t in range(2)]
        semA1 = ctx.enter_context(nc.semaphore("semA1"))
        semA2 = ctx.enter_context(nc.semaphore("semA2"))
        semA3 = ctx.enter_context(nc.semaphore("semA3"))
        semB1 = ctx.enter_context(nc.semaphore("semB1"))
        semB2 = ctx.enter_context(nc.semaphore("semB2"))
        semB3 = ctx.enter_context(nc.semaphore("semB3"))
        sem_br = ctx.enter_context(nc.semaphore("sem_br"))
        sem_bv = ctx.enter_context(nc.semaphore("sem_bv"))
        sem_warm = ctx.enter_context(nc.semaphore("sem_warm"))
        sem_mm = ctx.enter_context(nc.semaphore("sem_mm"))
        sem_act = ctx.enter_context(nc.semaphore("sem_act"))
        sem_v = ctx.enter_context(nc.semaphore("sem_v"))
        sem_o = ctx.enter_context(nc.semaphore("sem_o"))
        sem_o2 = ctx.enter_context(nc.semaphore("sem_o2"))
        block = ctx.enter_context(nc.Block())

        # W3 energy rows reuse z2[t] bank 0 partition 0 (written after the
        # L2 EXP drains z2[t])
        er = [z2[t][0:1, 0:SC] for t in range(2)]

        def w2s(h, k):
            return wb[:, (h * 2 + k) * 128 : (h * 2 + k + 1) * 128]

        def w3s(k):
            return wb[:, 512 + k : 513 + k]

        # x column ranges within f8sb for chunk t
        CUT1 = 768 + 2 * SC  # end of [wtA | x01 chunk0]
        CUT = 768 + XC  # end of [wtA | x chunk0]

        def x01(t):  # d0|d1 blocked halves, DoubleRow moving operand [128,2,SC]
            base = 768 + t * XC
            return f8sb[:, base : base + 2 * SC].rearrange("p (two f) -> p two f", two=2)

        def w1dr(h):  # DoubleRow stationary [128,2,128]
            return f8sb[:, h * 256 : (h + 1) * 256].rearrange("p (two f) -> p two f", two=2)

        def xd2(t):
            base = 768 + t * XC + 2 * SC
            return f8sb[:, base : base + SC]

        @block.sync
        def _(sync):
            sync.dma_start(f8sb[0:64, 0:CUT1], f8_ext[0:64, 0:CUT1]).then_inc(semA1, 16)
            sync.dma_start(f8sb[0:64, CUT1:CUT], f8_ext[0:64, CUT1:CUT]).then_inc(semA1, 16)
            sync.dma_start(wb[0:64, :], wb_ext[0:64, :]).then_inc(semA2, 16)
            sync.dma_start(brow_sb[:], brow_ext[:]).then_inc(sem_br, 16)
            sync.dma_start(f8sb[0:64, CUT:F8C], f8_ext[0:64, CUT:F8C]).then_inc(semA3, 16)
            sync.wait_ge(sem_v, 1)
            sync.dma_start(out_ext[0:1, 0:SC], out_sb[0:1, 0:SC]).then_inc(sem_o, 16)
            sync.wait_ge(sem_o, 16)

        @block.scalar
        def _(scalar):
            scalar.dma_start(f8sb[64:128, 0:CUT1], f8_ext[64:128, 0:CUT1]).then_inc(semB1, 16)
            scalar.dma_start(f8sb[64:128, CUT1:CUT], f8_ext[64:128, CUT1:CUT]).then_inc(semB1, 16)
            scalar.dma_start(wb[64:128, :], wb_ext[64:128, :]).then_inc(semB2, 16)
            scalar.dma_start(bv[:], bv_ext[:]).then_inc(sem_bv, 16)
            scalar.dma_start(f8sb[64:128, CUT:F8C], f8_ext[64:128, CUT:F8C]).then_inc(semB3, 16)
            # memzero lowers to an ACTIVATE, anchoring the ACT table load
            # before any cross-engine waits
            scalar.memzero(scratch[:])
            # sem_mm: z1t0h0=1, z1t0h1=2, z1t1=3, z2t0=4, z2t1=5, er0=6, er1=7
            # sem_act: e_h0=1, e_h1=2, ln1t0=3 | e=4, ln=5 | e=6, ln=7 |
            #          e=8, ln_k0=9, ln_k1=10
            scalar.wait_ge(semA2, 16)
            scalar.wait_ge(semB2, 16)
            scalar.wait_ge(sem_mm, 1)
            scalar.activation(
                t1f[0][:, 0:SC], z1[0][:, 0:SC], EXP, bias=wb[:, 514:515]
            ).then_inc(sem_act, 1)
            scalar.wait_ge(sem_mm, 2)
            scalar.activation(
                t1f[0][:, 512 : 512 + SC], z1[0][:, 512 : 512 + SC], EXP, bias=wb[:, 515:516]
            ).then_inc(sem_act, 1)
            scalar.wait_ge(sem_act, 2)  # ACT pipeline RAW
            scalar.activation(a1[0][:], t1f[0][:], LN, bias=1.0).then_inc(sem_act, 1)
            for li, zz, tt, aa in ((1, z1, t1f, a1), (2, z2, t2f, a2), (3, z2, t2f, a2)):
                t = li % 2
                scalar.wait_ge(sem_mm, li + 2)
                scalar.activation(tt[t][:], zz[t][:], EXP, bias=0.0).then_inc(sem_act, 1)
                scalar.wait_ge(sem_act, 2 * li + 2)  # exp fully written
                if li < 3:
                    scalar.activation(aa[t][:], tt[t][:], LN, bias=1.0).then_inc(sem_act, 1)
                else:
                    # k-split the last LN so er(t1) k0 overlaps ln k1
                    for k in range(2):
                        scalar.activation(
                            aa[t][:, k * 512 : k * 512 + 512],
                            tt[t][:, k * 512 : k * 512 + 512],
                            LN,
                            bias=1.0,
                        ).then_inc(sem_act, 1)
            scalar.wait_ge(sem_v, 2)
            scalar.dma_start(out_ext[0:1, SC:SV], out_sb[0:1, SC:SV]).then_inc(sem_o2, 16)
            scalar.wait_ge(sem_o2, 16)

        @block.tensor
        def _(tensor):
            tensor.wait_ge(sem_warm, 1)
            for _ in range(N_WARM):
                tensor.matmul(
                    z2[1][:, 0:512], warm[:, 0:128], warm[:], start=True, stop=True,
                    skip_group_check=True,
                )
            tensor.wait_ge(semA1, 16)
            tensor.wait_ge(semB1, 16)
            # L1 chunk 0: fp8 DoubleRow (d0|d1) + plain fp8 d2; bias via ACT.
            # DR h0 runs off the first (wtA+x01) transfer; d2 needs the second.
            tensor.matmul(
                z1[0][:, 0:SC], w1dr(0), x01(0),
                start=True, stop=False, perf_mode=DR, skip_group_check=True,
            )
            tensor.wait_ge(semA1, 32)
            tensor.wait_ge(semB1, 32)
            tensor.matmul(
                z1[0][:, 0:SC], f8sb[:, 512:640], xd2(0),
                start=False, stop=True, skip_group_check=True,
            ).then_inc(sem_mm, 1)
            tensor.matmul(
                z1[0][:, 512 : 512 + SC], w1dr(1), x01(0),
                start=True, stop=False, perf_mode=DR, skip_group_check=True,
            )
            tensor.matmul(
                z1[0][:, 512 : 512 + SC], f8sb[:, 640:768], xd2(0),
                start=False, stop=True, skip_group_check=True,
            ).then_inc(sem_mm, 1)
            # L1 chunk 1: bias via rank-1 ones-row matmuls
            tensor.wait_ge(semA3, 16)
            tensor.wait_ge(semB3, 16)
            tensor.wait_ge(sem_br, 16)
            tensor.wait_ge(sem_warm, 2)
            for h in range(2):
                tensor.matmul(
                    z1[1][:, h * 512 : h * 512 + SC],
                    w1dr(h),
                    x01(1),
                    start=True,
                    stop=False,
                    perf_mode=DR,
                    skip_group_check=True,
                )
                tensor.matmul(
                    z1[1][:, h * 512 : h * 512 + SC],
                    f8sb[:, 512 + h * 128 : 640 + h * 128],
                    xd2(1),
                    start=False,
                    stop=False,
                    skip_group_check=True,
                )
                mm = tensor.matmul(
                    z1[1][:, h * 512 : h * 512 + SC],
                    brow_sb[0:1, h * 128 : (h + 1) * 128],
                    ones[:],
                    start=False,
                    stop=True,
                    skip_group_check=True,
                )
            mm.then_inc(sem_mm, 1)

            def l2(t):
                for k in range(2):
                    for h in range(2):
                        tensor.matmul(
                            z2[t][:, k * 512 : k * 512 + SC],
                            w2s(h, k),
                            a1[t][:, h * 512 : h * 512 + SC],
                            start=(h == 0),
                            stop=False,
                            skip_group_check=True,
                        )
                    mm = tensor.matmul(
                        z2[t][:, k * 512 : k * 512 + SC],
                        brow_sb[0:1, 256 + k * 128 : 384 + k * 128],
                        ones[:],
                        start=False,
                        stop=True,
                        skip_group_check=True,
                    )
                mm.then_inc(sem_mm, 1)

            def l3(t, act_waits):
                for k in range(2):
                    tensor.wait_ge(sem_act, act_waits[k])
                    mm = tensor.matmul(
                        er[t],
                        w3s(k),
                        a2[t][:, k * 512 : k * 512 + SC],
                        start=(k == 0),
                        stop=(k == 1),
                        skip_group_check=True,
                    )
                mm.then_inc(sem_mm, 1)

            tensor.wait_ge(sem_act, 3)
            l2(0)  # -> 4
            # keep the HAM activity window fed through the LN(t1) wait so the
            # PE stays at 2.4GHz for l2(1) and the er tail
            for _ in range(3):
                tensor.matmul(
                    z2[1][:, 512:640], warm[:, 0:128], warm[:, 0:128],
                    start=True, stop=True, skip_group_check=True,
                )
            tensor.wait_ge(sem_act, 5)
            l2(1)  # -> 5
            l3(0, (7, 7))  # -> 6
            for _ in range(3):
                tensor.matmul(
                    z1[0][:, 0:256], warm[:, 0:128], warm[:, 0:256],
                    start=True, stop=True, skip_group_check=True,
                )
            l3(1, (9, 10))  # -> 7

        @block.vector
        def _(vector):
            vector.memzero(warm[:]).then_inc(sem_warm, 1)
            vector.wait_ge(sem_warm, 1)
            vector.tensor_scalar_add(ones[:], warm[0:1, 0:SC], 1.0).then_inc(sem_warm, 1)
            vector.wait_ge(sem_bv, 16)
            vector.wait_ge(sem_mm, 6)
            vector.tensor_scalar_add(
                out_sb[0:1, 0:SC], er[0], bv[0:1, 0:1]
            ).then_inc(sem_v, 1)
            vector.wait_ge(sem_mm, 7)
            vector.tensor_scalar_add(
                out_sb[0:1, SC:SV], er[1], bv[0:1, 0:1]
            ).then_inc(sem_v, 1)

    nc.finalize()
    return nc


def _build_generic(S: int):
    """Fallback graph for pathological element distributions (count > 1856):
    simple sequential per-chunk schedule, PSUM/SBUF reused across chunks."""
    from contextlib import ExitStack

    nc = _OneActSetBacc(None, target_bir_lowering=False)

    x_ext = nc.declare_dram_parameter("x", [128, 3 * S], BF16, isOutput=False)
    wt_ext = nc.declare_dram_parameter("wt", [128, 1282], BF16, isOutput=False)
    bias_ext = nc.declare_dram_parameter("bias", [128, 5], F32, isOutput=False)
    out_ext = nc.declare_dram_parameter("out", [1, S], F32, isOutput=True)

    EXP = mybir.ActivationFunctionType.Exp
    LN = mybir.ActivationFunctionType.Ln
    TCH = S // NT

    with ExitStack() as ctx:
        xt = ctx.enter_context(nc.sbuf_tensor([128, 3 * S], BF16))
        wt = ctx.enter_context(nc.sbuf_tensor([128, 1282], BF16))
        bias = ctx.enter_context(nc.sbuf_tensor([128, 5], F32))
        scratch = ctx.enter_context(nc.sbuf_tensor([1, 16], F32))
        out_sb = ctx.enter_context(nc.sbuf_tensor([1, S], F32))
        t1 = ctx.enter_context(nc.sbuf_tensor([128, 2 * NT], F32))
        a1 = ctx.enter_context(nc.sbuf_tensor([128, 2 * NT], BF16))
        t2 = ctx.enter_context(nc.sbuf_tensor([128, 2 * NT], F32))
        a2 = ctx.enter_context(nc.sbuf_tensor([128, 2 * NT], BF16))
        z1 = ctx.enter_context(nc.psum_tensor([128, 2 * NT], F32))
        z2 = ctx.enter_context(nc.psum_tensor([128, 2 * NT], F32))
        sem_xa = ctx.enter_context(nc.semaphore("sem_xa"))
        sem_xb = ctx.enter_context(nc.semaphore("sem_xb"))
        sem_w = ctx.enter_context(nc.semaphore("sem_w"))
        sem_b = ctx.enter_context(nc.semaphore("sem_b"))
        sem_o = ctx.enter_context(nc.semaphore("sem_o"))
        sem_o2 = ctx.enter_context(nc.semaphore("sem_o2"))
        sem_mm = ctx.enter_context(nc.semaphore("sem_mm"))
        sem_act = ctx.enter_context(nc.semaphore("sem_act"))
        sem_v = ctx.enter_context(nc.semaphore("sem_v"))
        block = ctx.enter_context(nc.Block())

        er = z2[0:1, 0:NT]

        def w1s(d, h):
            return wt[:, (d * 2 + h) * 128 : (d * 2 + h + 1) * 128]

        def w2s(h, k):
            return wt[:, 768 + (h * 2 + k) * 128 : 768 + (h * 2 + k + 1) * 128]

        def w3s(k):
            return wt[:, 1280 + k : 1281 + k]

        n_sync_outs = (TCH + 1) // 2
        n_scalar_outs = TCH // 2

        @block.sync
        def _(sync):
            for t in range(TCH):
                c = 3 * t * NT
                sync.dma_start(xt[:, c : c + 2 * NT], x_ext[:, c : c + 2 * NT]).then_inc(sem_xa, 16)
            sync.dma_start(bias[:], bias_ext[:]).then_inc(sem_b, 16)
            for i, t in enumerate(range(0, TCH, 2)):
                sync.wait_ge(sem_v, t + 1)
                sync.dma_start(out_ext[:, t * NT : (t + 1) * NT], out_sb[:, t * NT : (t + 1) * NT]).then_inc(sem_o, 16)
            sync.wait_ge(sem_o, 16 * n_sync_outs)

        @block.scalar
        def _(scalar):
            scalar.dma_start(wt[:], wt_ext[:]).then_inc(sem_w, 16)
            for t in range(TCH):
                c = (3 * t + 2) * NT
                scalar.dma_start(xt[:, c : c + NT], x_ext[:, c : c + NT]).then_inc(sem_xb, 16)
            scalar.memzero(scratch[:])
            scalar.wait_ge(sem_b, 16)
            # per t: mm incs z1=3t+1, z2=3t+2, er=3t+3; act incs 6 per t
            for t in range(TCH):
                scalar.wait_ge(sem_mm, 3 * t + 1)
                for h in range(2):
                    scalar.activation(
                        t1[:, h * NT : (h + 1) * NT], z1[:, h * NT : (h + 1) * NT],
                        EXP, bias=bias[:, h : h + 1],
                    ).then_inc(sem_act, 1)
                scalar.wait_ge(sem_act, 6 * t + 2)
                scalar.activation(a1[:], t1[:], LN, bias=1.0).then_inc(sem_act, 1)
                scalar.wait_ge(sem_mm, 3 * t + 2)
                for k in range(2):
                    scalar.activation(
                        t2[:, k * NT : (k + 1) * NT], z2[:, k * NT : (k + 1) * NT],
                        EXP, bias=bias[:, 2 + k : 3 + k],
                    ).then_inc(sem_act, 1)
                scalar.wait_ge(sem_act, 6 * t + 5)
                scalar.activation(a2[:], t2[:], LN, bias=1.0).then_inc(sem_act, 1)
            for i, t in enumerate(range(1, TCH, 2)):
                scalar.wait_ge(sem_v, t + 1)
                scalar.dma_start(out_ext[:, t * NT : (t + 1) * NT], out_sb[:, t * NT : (t + 1) * NT]).then_inc(sem_o2, 16)
            if n_scalar_outs:
                scalar.wait_ge(sem_o2, 16 * n_scalar_outs)

        @block.tensor
        def _(tensor):
            tensor.wait_ge(sem_w, 16)
            tensor.wait_ge(sem_xa, 16 * TCH)
            tensor.wait_ge(sem_xb, 16 * TCH)
            for t in range(TCH):
                if t > 0:
                    # z1 reused: exps of chunk t-1 must have drained it
                    tensor.wait_ge(sem_act, 6 * (t - 1) + 2)
                for h in range(2):
                    for d in range(3):
                        mm = tensor.matmul(
                            z1[:, h * NT : (h + 1) * NT], w1s(d, h),
                            xt[:, (t * 3 + d) * NT : (t * 3 + d + 1) * NT],
                            start=(d == 0), stop=(d == 2), skip_group_check=True,
                        )
                mm.then_inc(sem_mm, 1)
                tensor.wait_ge(sem_act, 6 * t + 3)
                if t > 0:
                    # z2 reused: er row of t-1 must be consumed by DVE
                    tensor.wait_ge(sem_v, t)
                for k in range(2):
                    for h in range(2):
                        mm = tensor.matmul(
                            z2[:, k * NT : (k + 1) * NT], w2s(h, k),
                            a1[:, h * NT : (h + 1) * NT],
                            start=(h == 0), stop=(h == 1), skip_group_check=True,
                        )
                mm.then_inc(sem_mm, 1)
                tensor.wait_ge(sem_act, 6 * t + 6)
                for k in range(2):
                    mm = tensor.matmul(
                        er, w3s(k), a2[:, k * NT : (k + 1) * NT],
                        start=(k == 0), stop=(k == 1), skip_group_check=True,
                    )
                mm.then_inc(sem_mm, 1)

        @block.vector
        def _(vector):
            for t in range(TCH):
                vector.wait_ge(sem_mm, 3 * t + 3)
                vector.tensor_scalar_add(
                    out_sb[:, t * NT : (t + 1) * NT], er, bias[0:1, 4:5]
                ).then_inc(sem_v, 1)

    nc.finalize()
    return nc


def _kernel_generic(flat, idxs, W1, b1, W2, b2_eff, W3, b3_eff, S, kwargs):
    """bf16 fallback path (original baseline device layout)."""
    bf16 = ml_dtypes.bfloat16
    Dd = flat.shape[1]
    key = ("gen", S)
    if key not in _CACHE:
        _CACHE[key] = _build_generic(S)
    nc = _CACHE[key]

    in_maps = []
    for c in range(N_CORES):
        e, half = divmod(c, 2)
        ix = idxs[e]
        lo = half * S
        hi = min(len(ix), lo + S)
        xs = np.zeros((S, Dd), np.float32)
        if hi > lo:
            xs[: hi - lo] = flat[ix[lo:hi]]
        wt = np.zeros((128, 1282), np.float32)
        wt[:, 0:768] = W1[e].reshape(3, 128, 2, 128).transpose(1, 0, 2, 3).reshape(128, 768)
        wt[:, 768:1280] = W2[e].reshape(2, 128, 2, 128).transpose(1, 0, 2, 3).reshape(128, 512)
        wt[:, 1280:1282] = W3[e].reshape(2, 128).T
        bias = np.zeros((128, 5), np.float32)
        bias[:, 0:2] = b1[e].reshape(2, 128).T
        bias[:, 2:4] = b2_eff[e].reshape(2, 128).T
        bias[0, 4] = b3_eff[e]
        in_maps.append(
            {
                "x": np.ascontiguousarray(
                    xs.T.reshape(3, 128, S // NT, NT).transpose(1, 2, 0, 3).reshape(128, 3 * S)
                ).astype(bf16),
                "wt": wt.astype(bf16),
                "bias": bias,
            }
        )
    res = run_bass_kernel_spmd(nc, in_maps, core_ids=list(range(N_CORES)), **kwargs)
    return res, S


def kernel(representation, atomic_numbers, elements, W1, b1, W2, b2, W3, b3):
    global LAST_EXEC_NS
    rep = np.asarray(representation, dtype=np.float32)
    an = np.asarray(atomic_numbers).astype(np.int64)
    el = np.asarray(elements).astype(np.int64)
    W1 = np.asarray(W1, dtype=np.float32)
    b1 = np.asarray(b1, dtype=np.float32)
    W2 = np.asarray(W2, dtype=np.float32)
    b2 = np.asarray(b2, dtype=np.float32)
    W3 = np.asarray(W3, dtype=np.float32)
    b3 = np.asarray(b3, dtype=np.float32)

    Bsz, Nn, Dd = rep.shape
    flat = rep.reshape(-1, Dd)
    anf = an.reshape(-1)

    idxs = [np.nonzero(anf == el[e])[0] for e in range(E)]
    counts = [len(ix) for ix in idxs]

    # fold the shifted-softplus -log(2) into downstream biases
    b2_eff = b2 - LOG2 * W2.sum(axis=1)  # [E, H2]
    b3_eff = b3 - LOG2 * W3.sum(axis=1)  # [E]

    kwargs = {}
    if PROFILE:
        kwargs = dict(trace=True, trace_cores=list(TRACE_CORES))

    if max(counts) > 2 * SV:
        # pathological distribution: legacy bf16 path with big capacity
        S = 1024
        while max(counts) > 2 * S:
            S *= 2
        res, S_used = _kernel_generic(flat, idxs, W1, b1, W2, b2_eff, W3, b3_eff, S, kwargs)
    else:
        S_used = SV
        bf16 = ml_dtypes.bfloat16
        bf8 = ml_dtypes.float8_e4m3
        if "v2" not in _CACHE:
            _CACHE["v2"] = _build_v2()
        nc = _CACHE["v2"]

        in_maps = []
        for c in range(N_CORES):
            e, half = divmod(c, 2)
            ix = idxs[e]
            lo = half * SV
            hi = min(len(ix), lo + SV)
            xs = np.zeros((SV, Dd), np.float32)
            if hi > lo:
                xs[: hi - lo] = flat[ix[lo:hi]]
            # x: per chunk [d0|d1|d2] tiles of SC slot-columns each
            xb = xs.T.reshape(3, 128, 2, SC).transpose(1, 2, 0, 3).reshape(128, 2 * XC)
            W1e = W1[e]
            wtA = np.empty((128, 768), np.float32)
            for h in range(2):
                wtA[:, h * 256 : h * 256 + 128] = W1e[0:128, h * 128 : (h + 1) * 128]
                wtA[:, h * 256 + 128 : h * 256 + 256] = W1e[128:256, h * 128 : (h + 1) * 128]
                wtA[:, 512 + h * 128 : 640 + h * 128] = W1e[256:384, h * 128 : (h + 1) * 128]
            f8 = np.concatenate([wtA, xb], axis=1).astype(bf8)
            wb = np.zeros((128, 516), np.float32)
            for h in range(2):
                for k in range(2):
                    wb[:, (h * 2 + k) * 128 : (h * 2 + k + 1) * 128] = (
                        W2[e][h * 128 : (h + 1) * 128, k * 128 : (k + 1) * 128]
                    )
            wb[:, 512:514] = W3[e].reshape(2, 128).T
            wb[:, 514] = b1[e][0:128]
            wb[:, 515] = b1[e][128:256]
            bv = np.zeros((1, 8), np.float32)
            bv[0, 0] = b3_eff[e]
            in_maps.append(
                {
                    "f8": np.ascontiguousarray(f8),
                    "wb": wb.astype(bf16),
                    "brow": np.concatenate([b1[e], b2_eff[e]]).reshape(1, 512).astype(bf16),
                    "bv": bv,
                }
            )
        res = run_bass_kernel_spmd(nc, in_maps, core_ids=list(range(N_CORES)), **kwargs)

    LAST_EXEC_NS = res.exec_time_ns

    energies = np.zeros(Bsz, np.float64)
    for c in range(N_CORES):
        e, half = divmod(c, 2)
        ix = idxs[e]
        lo = half * S_used
        hi = min(len(ix), lo + S_used)
        if hi <= lo:
            continue
        evals = np.asarray(res.results[c]["out"]).reshape(-1)[: hi - lo]
        np.add.at(energies, ix[lo:hi] // Nn, evals.astype(np.float64))
    return energies.astype(np.float32)


# revision 19
# speedup vs baseline: 1.1004x; 1.1004x over previous
"""ANI-style element-MLP (MoE routing) kernel for 8 TRN2 NeuronCores.

Strategy (v2):
  - Host: bucket atoms by element (expert); cores 2e, 2e+1 own expert e,
    928 slots each (capacity 1856/expert covers the ~1850 max count).
    Per-core inputs are packed into two byte-blobs so the whole input
    lands in 6 logical DMAs (3 per HWDGE ring, partition-split 0:64 /
    64:128 so the two rings use disjoint SDMA engines):
      f8 blob  (fp8 e4m3): W1 tiles | x chunk0 | x chunk1
      wb blob  (bf16)    : W2 tiles | W3 cols | b1 / b3 bias cols
      brow     (bf16)    : [1,512] b1|b2_eff row for rank-1 bias matmuls
  - Device: L1 runs in fp8 with DoubleRow (K=256 per pass: d0|d1 blocked
    halves), d2 as a plain fp8 K=128 pass.  L2/L3 in bf16.  Softplus =
    EXP then LN(1+t) on the ACT engine; chunk-0 L1 EXP is h-split with
    the per-partition ACT bias (saves the cold-PE bias matmuls on the
    critical path), all other biases ride rank-1 ones-row matmuls.
    The -log(2) shift is folded into downstream biases on host.
  - PE warmup matmuls run before the x DMA lands to keep the HAM clock
    monitor fed (2.4 GHz boost).
  - Host: scatter-add real slots' energies into the per-molecule output.

Self-contained: hardcodes problem shapes B=32, N=512, D=384, E=4, H=256.
"""

import ml_dtypes
import numpy as np

import concourse.bass as bass  # noqa: F401  (bass types referenced via bacc/mybir)
import concourse.mybir as mybir
from concourse import bacc
from concourse.bass_utils import run_bass_kernel_spmd
from concourse.hw_specs import get_activation_tables

class _OneActSetBacc(bacc.Bacc):
    """All our ACT functions (Exp, Ln, Identity) live in the
    natural_log_exp_and_others table set, but the stock table-load pass
    assigns each function its first matching set, thrashing ~1.5us table
    loads between sets on every layer.  Force every load to the one set
    that covers all three and drop the now-redundant reloads."""

    _ACT_SET = "natural_log_exp_and_others"

    def insert_act_table_loads(self):
        super().insert_act_table_loads()
        names = list(get_activation_tables(self.m.arch))
        target = names.index(self._ACT_SET)
        for blk in self.main_func.blocks:
            seen_engines = set()
            to_remove = []
            for inst in blk.instructions:
                if isinstance(inst, mybir.InstLoadActFuncSet):
                    if inst.engine in seen_engines and not (inst.has_wait() or inst.has_update()):
                        to_remove.append(inst)
                    else:
                        inst.act_func_set_id = target
                        seen_engines.add(inst.engine)
            for inst in to_remove:
                blk.instructions.remove(inst)

LOG2 = np.float32(np.log(2.0))
B, N, D = 32, 512, 384
E = 4
H1 = H2 = 256
N_CORES = 8
NT = 512   # legacy fallback tile
SC = 480   # v2 slots per chunk (2 chunks per core); capacity 4*SC per expert
SV = 2 * SC  # 960 slots per core
XC = 3 * SC  # x columns per chunk (d0|d1|d2 tiles)
F8C = 768 + 2 * XC  # f8 blob columns: wtA | x chunk0 | x chunk1

F32 = mybir.dt.float32
BF16 = mybir.dt.bfloat16
F8 = mybir.dt.float8e4

# Set by test harnesses: PROFILE=True makes kernel() run with NTFF tracing and
# store the profiled NEFF exec time (ns) in LAST_EXEC_NS.
PROFILE = False
TRACE_CORES = [0]
LAST_EXEC_NS = None

_CACHE: dict = {}

N_WARM = 7  # PE warmup matmuls issued while the x DMA streams in


def _build_v2():
    """Raw-Bass per-core graph, 928 slots (2 chunks of 464), fp8 L1.

    Engine plan (explicit semaphores, no Tile):
      sync   : ring A DMAs (partitions 0:64 of both blobs + brow), out t0
      scalar : ring B DMAs (partitions 64:128), all EXP/LN, out t1
      tensor : warmup + L1 (fp8 DoubleRow) + L2/L3 (bf16) + bias matmuls
      vector : warm/ones setup, +b3 epilogues PSUM->SBUF
    """
    from contextlib import ExitStack

    nc = _OneActSetBacc(None, target_bir_lowering=False)

    # f8 blob: wtA [128,768] | x chunk0 [128,XC] | x chunk1 [128,XC]
    f8_ext = nc.declare_dram_parameter("f8", [128, F8C], F8, isOutput=False)
    # wb blob: W2 tiles 512 | W3 2 | b1h0 b1h1 (ACT bias) 2
    wb_ext = nc.declare_dram_parameter("wb", [128, 516], BF16, isOutput=False)
    brow_ext = nc.declare_dram_parameter("brow", [1, 512], BF16, isOutput=False)
    bv_ext = nc.declare_dram_parameter("bv", [1, 8], F32, isOutput=False)
    out_ext = nc.declare_dram_parameter("out", [1, SV], F32, isOutput=True)

    EXP = mybir.ActivationFunctionType.Exp
    LN = mybir.ActivationFunctionType.Ln
    DR = mybir.MatmulPerfMode.DoubleRow

    with ExitStack() as ctx:
        f8sb = ctx.enter_context(nc.sbuf_tensor([128, F8C], F8))
        wb = ctx.enter_context(nc.sbuf_tensor([128, 516], BF16))
        bv = ctx.enter_context(nc.sbuf_tensor([1, 8], F32))
        brow_sb = ctx.enter_context(nc.sbuf_tensor([1, 512], BF16))
        warm = ctx.enter_context(nc.sbuf_tensor([128, 512], BF16))
        ones = ctx.enter_context(nc.sbuf_tensor([1, SC], BF16))
        scratch = ctx.enter_context(nc.sbuf_tensor([1, 16], F32))
        out_sb = ctx.enter_context(nc.sbuf_tensor([1, SV], F32))
        t1f = [ctx.enter_context(nc.sbuf_tensor(f"t1f_{t}", [128, 1024], F32)) for t in range(2)]
        a1 = [ctx.enter_context(nc.sbuf_tensor(f"a1_{t}", [128, 1024], BF16)) for t in range(2)]
        t2f = [ctx.enter_context(nc.sbuf_tensor(f"t2f_{t}", [128, 1024], F32)) for t in range(2)]
        a2 = [ctx.enter_context(nc.sbuf_tensor(f"a2_{t}", [128, 1024], BF16)) for t in range(2)]
        z1 = [ctx.enter_context(nc.psum_tensor(f"z1_{t}", [128, 1024], F32)) for t in range(2)]
        z2 = [ctx.enter_context(nc.psum_tensor(f"z2_{t}", [128, 1024], F32)) for t in range(2)]
        semA1 = ctx.enter_context(nc.semaphore("semA1"))
        semA2 = ctx.enter_context(nc.semaphore("semA2"))
        semA3 = ctx.enter_context(nc.semaphore("semA3"))
        semB1 = ctx.enter_context(nc.semaphore("semB1"))
        semB2 = ctx.enter_context(nc.semaphore("semB2"))
        semB3 = ctx.enter_context(nc.semaphore("semB3"))
        sem_br = ctx.enter_context(nc.semaphore("sem_br"))
        sem_bv = ctx.enter_context(nc.semaphore("sem_bv"))
        semA1b = ctx.enter_context(nc.semaphore("semA1b"))
        semB1b = ctx.enter_context(nc.semaphore("semB1b"))
        sem_warm = ctx.enter_context(nc.semaphore("sem_warm"))
        sem_mm = ctx.enter_context(nc.semaphore("sem_mm"))
        sem_act = ctx.enter_context(nc.semaphore("sem_act"))
        sem_v = ctx.enter_context(nc.semaphore("sem_v"))
        sem_o = ctx.enter_context(nc.semaphore("sem_o"))
        sem_o2 = ctx.enter_context(nc.semaphore("sem_o2"))
        block = ctx.enter_context(nc.Block())

        # W3 energy rows reuse z2[t] bank 0 partition 0 (written after the
        # L2 EXP drains z2[t])
        er = [z2[t][0:1, 0:SC] for t in range(2)]

        def w2s(h, k):
            return wb[:, (h * 2 + k) * 128 : (h * 2 + k + 1) * 128]

        def w3s(k):
            return wb[:, 512 + k : 513 + k]

        # x column ranges within f8sb for chunk t
        CUT1 = 768 + 2 * SC  # end of [wtA | x01 chunk0]
        CUT = 768 + XC  # end of [wtA | x chunk0]

        def x01(t):  # d0|d1 blocked halves, DoubleRow moving operand [128,2,SC]
            base = 768 + t * XC
            return f8sb[:, base : base + 2 * SC].rearrange("p (two f) -> p two f", two=2)

        def w1dr(h):  # DoubleRow stationary [128,2,128]
            return f8sb[:, h * 256 : (h + 1) * 256].rearrange("p (two f) -> p two f", two=2)

        def xd2(t):
            base = 768 + t * XC + 2 * SC
            return f8sb[:, base : base + SC]

        @block.sync
        def _(sync):
            sync.dma_start(f8sb[0:64, 0:CUT1], f8_ext[0:64, 0:CUT1]).then_inc(semA1, 16)
            sync.dma_start(f8sb[0:64, CUT1:CUT], f8_ext[0:64, CUT1:CUT]).then_inc(semA1b, 16)
            sync.dma_start(wb[0:64, :], wb_ext[0:64, :]).then_inc(semA2, 16)
            sync.dma_start(brow_sb[:], brow_ext[:]).then_inc(sem_br, 16)
            sync.dma_start(f8sb[0:64, CUT:F8C], f8_ext[0:64, CUT:F8C]).then_inc(semA3, 16)
            sync.wait_ge(sem_v, 1)
            sync.dma_start(out_ext[0:1, 0:SC], out_sb[0:1, 0:SC]).then_inc(sem_o, 16)
            sync.wait_ge(sem_o, 16)

        @block.scalar
        def _(scalar):
            scalar.dma_start(f8sb[64:128, 0:CUT1], f8_ext[64:128, 0:CUT1]).then_inc(semB1, 16)
            scalar.dma_start(f8sb[64:128, CUT1:CUT], f8_ext[64:128, CUT1:CUT]).then_inc(semB1b, 16)
            scalar.dma_start(wb[64:128, :], wb_ext[64:128, :]).then_inc(semB2, 16)
            scalar.dma_start(bv[:], bv_ext[:]).then_inc(sem_bv, 16)
            scalar.dma_start(f8sb[64:128, CUT:F8C], f8_ext[64:128, CUT:F8C]).then_inc(semB3, 16)
            # memzero lowers to an ACTIVATE, anchoring the ACT table load
            # before any cross-engine waits
            scalar.memzero(scratch[:])
            # sem_mm: z1t0h0=1, z1t0h1=2, z1t1=3, z2t0=4, z2t1=5, er0=6, er1=7
            # sem_act: e_h0=1, e_h1=2, ln1t0=3 | e=4, ln=5 | e=6, ln=7 |
            #          e=8, ln_k0=9, ln_k1=10
            scalar.wait_ge(semA2, 16)
            scalar.wait_ge(semB2, 16)
            scalar.wait_ge(sem_mm, 1)
            scalar.activation(
                t1f[0][:, 0:SC], z1[0][:, 0:SC], EXP, bias=wb[:, 514:515]
            ).then_inc(sem_act, 1)
            scalar.wait_ge(sem_mm, 2)
            scalar.activation(
                t1f[0][:, 512 : 512 + SC], z1[0][:, 512 : 512 + SC], EXP, bias=wb[:, 515:516]
            ).then_inc(sem_act, 1)
            scalar.wait_ge(sem_act, 2)  # ACT pipeline RAW
            scalar.activation(a1[0][:], t1f[0][:], LN, bias=1.0).then_inc(sem_act, 1)
            for li, zz, tt, aa in ((1, z1, t1f, a1), (2, z2, t2f, a2), (3, z2, t2f, a2)):
                t = li % 2
                scalar.wait_ge(sem_mm, li + 2)
                scalar.activation(tt[t][:], zz[t][:], EXP, bias=0.0).then_inc(sem_act, 1)
                scalar.wait_ge(sem_act, 2 * li + 2)  # exp fully written
                if li < 3:
                    scalar.activation(aa[t][:], tt[t][:], LN, bias=1.0).then_inc(sem_act, 1)
                else:
                    # k-split the last LN so er(t1) k0 overlaps ln k1
                    for k in range(2):
                        scalar.activation(
                            aa[t][:, k * 512 : k * 512 + 512],
                            tt[t][:, k * 512 : k * 512 + 512],
                            LN,
                            bias=1.0,
                        ).then_inc(sem_act, 1)
            scalar.wait_ge(sem_v, 2)
            scalar.dma_start(out_ext[0:1, SC:SV], out_sb[0:1, SC:SV]).then_inc(sem_o2, 16)
            scalar.wait_ge(sem_o2, 16)

        @block.tensor
        def _(tensor):
            tensor.wait_ge(sem_warm, 1)
            for _ in range(N_WARM):
                tensor.matmul(
                    z2[1][:, 0:512], warm[:, 0:128], warm[:], start=True, stop=True,
                    skip_group_check=True,
                )
            tensor.wait_ge(semA1, 16)
            tensor.wait_ge(semB1, 16)
            # L1 chunk 0: fp8 DoubleRow (d0|d1) + plain fp8 d2; bias via ACT.
            # DR h0 runs off the first (wtA+x01) transfer; d2 needs the second.
            tensor.matmul(
                z1[0][:, 0:SC], w1dr(0), x01(0),
                start=True, stop=False, perf_mode=DR, skip_group_check=True,
            )
            tensor.wait_ge(semA1b, 16)
            tensor.wait_ge(semB1b, 16)
            tensor.matmul(
                z1[0][:, 0:SC], f8sb[:, 512:640], xd2(0),
                start=False, stop=True, skip_group_check=True,
            ).then_inc(sem_mm, 1)
            tensor.matmul(
                z1[0][:, 512 : 512 + SC], w1dr(1), x01(0),
                start=True, stop=False, perf_mode=DR, skip_group_check=True,
            )
            tensor.matmul(
                z1[0][:, 512 : 512 + SC], f8sb[:, 640:768], xd2(0),
                start=False, stop=True, skip_group_check=True,
            ).then_inc(sem_mm, 1)
            # L1 chunk 1: bias via rank-1 ones-row matmuls
            tensor.wait_ge(semA3, 16)
            tensor.wait_ge(semB3, 16)
            tensor.wait_ge(sem_br, 16)
            tensor.wait_ge(sem_warm, 2)
            for h in range(2):
                tensor.matmul(
                    z1[1][:, h * 512 : h * 512 + SC],
                    w1dr(h),
                    x01(1),
                    start=True,
                    stop=False,
                    perf_mode=DR,
                    skip_group_check=True,
                )
                tensor.matmul(
                    z1[1][:, h * 512 : h * 512 + SC],
                    f8sb[:, 512 + h * 128 : 640 + h * 128],
                    xd2(1),
                    start=False,
                    stop=False,
                    skip_group_check=True,
                )
                mm = tensor.matmul(
                    z1[1][:, h * 512 : h * 512 + SC],
                    brow_sb[0:1, h * 128 : (h + 1) * 128],
                    ones[:],
                    start=False,
                    stop=True,
                    skip_group_check=True,
                )
            mm.then_inc(sem_mm, 1)

            def l2(t):
                for k in range(2):
                    for h in range(2):
                        tensor.matmul(
                            z2[t][:, k * 512 : k * 512 + SC],
                            w2s(h, k),
                            a1[t][:, h * 512 : h * 512 + SC],
                            start=(h == 0),
                            stop=False,
                            skip_group_check=True,
                        )
                    mm = tensor.matmul(
                        z2[t][:, k * 512 : k * 512 + SC],
                        brow_sb[0:1, 256 + k * 128 : 384 + k * 128],
                        ones[:],
                        start=False,
                        stop=True,
                        skip_group_check=True,
                    )
                mm.then_inc(sem_mm, 1)

            def l3(t, act_waits):
                for k in range(2):
                    tensor.wait_ge(sem_act, act_waits[k])
                    mm = tensor.matmul(
                        er[t],
                        w3s(k),
                        a2[t][:, k * 512 : k * 512 + SC],
                        start=(k == 0),
                        stop=(k == 1),
                        skip_group_check=True,
                    )
                mm.then_inc(sem_mm, 1)

            tensor.wait_ge(sem_act, 3)
            l2(0)  # -> 4
            # keep the HAM activity window fed through the LN(t1) wait so the
            # PE stays at 2.4GHz for l2(1) and the er tail
            for _ in range(3):
                tensor.matmul(
                    z2[1][:, 512:640], warm[:, 0:128], warm[:, 0:128],
                    start=True, stop=True, skip_group_check=True,
                )
            tensor.wait_ge(sem_act, 5)
            l2(1)  # -> 5
            l3(0, (7, 7))  # -> 6
            l3(1, (9, 10))  # -> 7

        @block.vector
        def _(vector):
            vector.memzero(warm[:]).then_inc(sem_warm, 1)
            vector.wait_ge(sem_warm, 1)
            vector.tensor_scalar_add(ones[:], warm[0:1, 0:SC], 1.0).then_inc(sem_warm, 1)
            vector.wait_ge(sem_bv, 16)
            vector.wait_ge(sem_mm, 6)
            vector.tensor_scalar_add(
                out_sb[0:1, 0:SC], er[0], bv[0:1, 0:1]
            ).then_inc(sem_v, 1)
            vector.wait_ge(sem_mm, 7)
            vector.tensor_scalar_add(
                out_sb[0:1, SC:SV], er[1], bv[0:1, 0:1]
            ).then_inc(sem_v, 1)

    nc.finalize()
    return nc


def _build_generic(S: int):
    """Fallback graph for pathological element distributions (count > 1856):
    simple sequential per-chunk schedule, PSUM/SBUF reused across chunks."""
    from contextlib import ExitStack

    nc = _OneActSetBacc(None, target_bir_lowering=False)

    x_ext = nc.declare_dram_parameter("x", [128, 3 * S], BF16, isOutput=False)
    wt_ext = nc.declare_dram_parameter("wt", [128, 1282], BF16, isOutput=False)
    bias_ext = nc.declare_dram_parameter("bias", [128, 5], F32, isOutput=False)
    out_ext = nc.declare_dram_parameter("out", [1, S], F32, isOutput=True)

    EXP = mybir.ActivationFunctionType.Exp
    LN = mybir.ActivationFunctionType.Ln
    TCH = S // NT

    with ExitStack() as ctx:
        xt = ctx.enter_context(nc.sbuf_tensor([128, 3 * S], BF16))
        wt = ctx.enter_context(nc.sbuf_tensor([128, 1282], BF16))
        bias = ctx.enter_context(nc.sbuf_tensor([128, 5], F32))
        scratch = ctx.enter_context(nc.sbuf_tensor([1, 16], F32))
        out_sb = ctx.enter_context(nc.sbuf_tensor([1, S], F32))
        t1 = ctx.enter_context(nc.sbuf_tensor([128, 2 * NT], F32))
        a1 = ctx.enter_context(nc.sbuf_tensor([128, 2 * NT], BF16))
        t2 = ctx.enter_context(nc.sbuf_tensor([128, 2 * NT], F32))
        a2 = ctx.enter_context(nc.sbuf_tensor([128, 2 * NT], BF16))
        z1 = ctx.enter_context(nc.psum_tensor([128, 2 * NT], F32))
        z2 = ctx.enter_context(nc.psum_tensor([128, 2 * NT], F32))
        sem_xa = ctx.enter_context(nc.semaphore("sem_xa"))
        sem_xb = ctx.enter_context(nc.semaphore("sem_xb"))
        sem_w = ctx.enter_context(nc.semaphore("sem_w"))
        sem_b = ctx.enter_context(nc.semaphore("sem_b"))
        sem_o = ctx.enter_context(nc.semaphore("sem_o"))
        sem_o2 = ctx.enter_context(nc.semaphore("sem_o2"))
        sem_mm = ctx.enter_context(nc.semaphore("sem_mm"))
        sem_act = ctx.enter_context(nc.semaphore("sem_act"))
        sem_v = ctx.enter_context(nc.semaphore("sem_v"))
        block = ctx.enter_context(nc.Block())

        er = z2[0:1, 0:NT]

        def w1s(d, h):
            return wt[:, (d * 2 + h) * 128 : (d * 2 + h + 1) * 128]

        def w2s(h, k):
            return wt[:, 768 + (h * 2 + k) * 128 : 768 + (h * 2 + k + 1) * 128]

        def w3s(k):
            return wt[:, 1280 + k : 1281 + k]

        n_sync_outs = (TCH + 1) // 2
        n_scalar_outs = TCH // 2

        @block.sync
        def _(sync):
            for t in range(TCH):
                c = 3 * t * NT
                sync.dma_start(xt[:, c : c + 2 * NT], x_ext[:, c : c + 2 * NT]).then_inc(sem_xa, 16)
            sync.dma_start(bias[:], bias_ext[:]).then_inc(sem_b, 16)
            for i, t in enumerate(range(0, TCH, 2)):
                sync.wait_ge(sem_v, t + 1)
                sync.dma_start(out_ext[:, t * NT : (t + 1) * NT], out_sb[:, t * NT : (t + 1) * NT]).then_inc(sem_o, 16)
            sync.wait_ge(sem_o, 16 * n_sync_outs)

        @block.scalar
        def _(scalar):
            scalar.dma_start(wt[:], wt_ext[:]).then_inc(sem_w, 16)
            for t in range(TCH):
                c = (3 * t + 2) * NT
                scalar.dma_start(xt[:, c : c + NT], x_ext[:, c : c + NT]).then_inc(sem_xb, 16)
            scalar.memzero(scratch[:])
            scalar.wait_ge(sem_b, 16)
            # per t: mm incs z1=3t+1, z2=3t+2, er=3t+3; act incs 6 per t
            for t in range(TCH):
                scalar.wait_ge(sem_mm, 3 * t + 1)
                for h in range(2):
                    scalar.activation(
                        t1[:, h * NT : (h + 1) * NT], z1[:, h * NT : (h + 1) * NT],
                        EXP, bias=bias[:, h : h + 1],
                    ).then_inc(sem_act, 1)
                scalar.wait_ge(sem_act, 6 * t + 2)
                scalar.activation(a1[:], t1[:], LN, bias=1.0).then_inc(sem_act, 1)
                scalar.wait_ge(sem_mm, 3 * t + 2)
                for k in range(2):
                    scalar.activation(
                        t2[:, k * NT : (k + 1) * NT], z2[:, k * NT : (k + 1) * NT],
                        EXP, bias=bias[:, 2 + k : 3 + k],
                    ).then_inc(sem_act, 1)
                scalar.wait_ge(sem_act, 6 * t + 5)
                scalar.activation(a2[:], t2[:], LN, bias=1.0).then_inc(sem_act, 1)
            for i, t in enumerate(range(1, TCH, 2)):
                scalar.wait_ge(sem_v, t + 1)
                scalar.dma_start(out_ext[:, t * NT : (t + 1) * NT], out_sb[:, t * NT : (t + 1) * NT]).then_inc(sem_o2, 16)
            if n_scalar_outs:
                scalar.wait_ge(sem_o2, 16 * n_scalar_outs)

        @block.tensor
        def _(tensor):
            tensor.wait_ge(sem_w, 16)
            tensor.wait_ge(sem_xa, 16 * TCH)
            tensor.wait_ge(sem_xb, 16 * TCH)
            for t in range(TCH):
                if t > 0:
                    # z1 reused: exps of chunk t-1 must have drained it
                    tensor.wait_ge(sem_act, 6 * (t - 1) + 2)
                for h in range(2):
                    for d in range(3):
                        mm = tensor.matmul(
                            z1[:, h * NT : (h + 1) * NT], w1s(d, h),
                            xt[:, (t * 3 + d) * NT : (t * 3 + d + 1) * NT],
                            start=(d == 0), stop=(d == 2), skip_group_check=True,
                        )
                mm.then_inc(sem_mm, 1)
                tensor.wait_ge(sem_act, 6 * t + 3)
                if t > 0:
                    # z2 reused: er row of t-1 must be consumed by DVE
                    tensor.wait_ge(sem_v, t)
                for k in range(2):
                    for h in range(2):
                        mm = tensor.matmul(
                            z2[:, k * NT : (k + 1) * NT], w2s(h, k),
                            a1[:, h * NT : (h + 1) * NT],
                            start=(h == 0), stop=(h == 1), skip_group_check=True,
                        )
                mm.then_inc(sem_mm, 1)
                tensor.wait_ge(sem_act, 6 * t + 6)
                for k in range(2):
                    mm = tensor.matmul(
                        er, w3s(k), a2[:, k * NT : (k + 1) * NT],
                        start=(k == 0), stop=(k == 1), skip_group_check=True,
                    )
                mm.then_inc(sem_mm, 1)

        @block.vector
        def _(vector):
            for t in range(TCH):
                vector.wait_ge(sem_mm, 3 * t + 3)
                vector.tensor_scalar_add(
                    out_sb[:, t * NT : (t + 1) * NT], er, bias[0:1, 4:5]
                ).then_inc(sem_v, 1)

    nc.finalize()
    return nc


def _kernel_generic(flat, idxs, W1, b1, W2, b2_eff, W3, b3_eff, S, kwargs):
    """bf16 fallback path (original baseline device layout)."""
    bf16 = ml_dtypes.bfloat16
    Dd = flat.shape[1]
    key = ("gen", S)
    if key not in _CACHE:
        _CACHE[key] = _build_generic(S)
    nc = _CACHE[key]

    in_maps = []
    for c in range(N_CORES):
        e, half = divmod(c, 2)
        ix = idxs[e]
        lo = half * S
        hi = min(len(ix), lo + S)
        xs = np.zeros((S, Dd), np.float32)
        if hi > lo:
            xs[: hi - lo] = flat[ix[lo:hi]]
        wt = np.zeros((128, 1282), np.float32)
        wt[:, 0:768] = W1[e].reshape(3, 128, 2, 128).transpose(1, 0, 2, 3).reshape(128, 768)
        wt[:, 768:1280] = W2[e].reshape(2, 128, 2, 128).transpose(1, 0, 2, 3).reshape(128, 512)
        wt[:, 1280:1282] = W3[e].reshape(2, 128).T
        bias = np.zeros((128, 5), np.float32)
        bias[:, 0:2] = b1[e].reshape(2, 128).T
        bias[:, 2:4] = b2_eff[e].reshape(2, 128).T
        bias[0, 4] = b3_eff[e]
        in_maps.append(
            {
                "x": np.ascontiguousarray(
                    xs.T.reshape(3, 128, S // NT, NT).transpose(1, 2, 0, 3).reshape(128, 3 * S)
                ).astype(bf16),
                "wt": wt.astype(bf16),
                "bias": bias,
            }
        )
    res = run_bass_kernel_spmd(nc, in_maps, core_ids=list(range(N_CORES)), **kwargs)
    return res, S


def kernel(representation, atomic_numbers, elements, W1, b1, W2, b2, W3, b3):
    global LAST_EXEC_NS
    rep = np.asarray(representation, dtype=np.float32)
    an = np.asarray(atomic_numbers).astype(np.int64)
    el = np.asarray(elements).astype(np.int64)
    W1 = np.asarray(W1, dtype=np.float32)
    b1 = np.asarray(b1, dtype=np.float32)
    W2 = np.asarray(W2, dtype=np.float32)
    b2 = np.asarray(b2, dtype=np.float32)
    W3 = np.asarray(W3, dtype=np.float32)
    b3 = np.asarray(b3, dtype=np.float32)

    Bsz, Nn, Dd = rep.shape
    flat = rep.reshape(-1, Dd)
    anf = an.reshape(-1)

    idxs = [np.nonzero(anf == el[e])[0] for e in range(E)]
    counts = [len(ix) for ix in idxs]

    # fold the shifted-softplus -log(2) into downstream biases
    b2_eff = b2 - LOG2 * W2.sum(axis=1)  # [E, H2]
    b3_eff = b3 - LOG2 * W3.sum(axis=1)  # [E]

    kwargs = {}
    if PROFILE:
        kwargs = dict(trace=True, trace_cores=list(TRACE_CORES))

    if max(counts) > 2 * SV:
        # pathological distribution: legacy bf16 path with big capacity
        S = 1024
        while max(counts) > 2 * S:
            S *= 2
        res, S_used = _kernel_generic(flat, idxs, W1, b1, W2, b2_eff, W3, b3_eff, S, kwargs)
    else:
        S_used = SV
        bf16 = ml_dtypes.bfloat16
        bf8 = ml_dtypes.float8_e4m3
        if "v2" not in _CACHE:
            _CACHE["v2"] = _build_v2()
        nc = _CACHE["v2"]

        in_maps = []
        for c in range(N_CORES):
            e, half = divmod(c, 2)
            ix = idxs[e]
            lo = half * SV
            hi = min(len(ix), lo + SV)
            xs = np.zeros((SV, Dd), np.float32)
            if hi > lo:
                xs[: hi - lo] = flat[ix[lo:hi]]
            # x: per chunk [d0|d1|d2] tiles of SC slot-columns each
            xb = xs.T.reshape(3, 128, 2, SC).transpose(1, 2, 0, 3).reshape(128, 2 * XC)
            W1e = W1[e]
            wtA = np.empty((128, 768), np.float32)
            for h in range(2):
                wtA[:, h * 256 : h * 256 + 128] = W1e[0:128, h * 128 : (h + 1) * 128]
                wtA[:, h * 256 + 128 : h * 256 + 256] = W1e[128:256, h * 128 : (h + 1) * 128]
                wtA[:, 512 + h * 128 : 640 + h * 128] = W1e[256:384, h * 128 : (h + 1) * 128]
            f8 = np.concatenate([wtA, xb], axis=1).astype(bf8)
            wb = np.zeros((128, 516), np.float32)
            for h in range(2):
                for k in range(2):
                    wb[:, (h * 2 + k) * 128 : (h * 2 + k + 1) * 128] = (
                        W2[e][h * 128 : (h + 1) * 128, k * 128 : (k + 1) * 128]
                    )
            wb[:, 512:514] = W3[e].reshape(2, 128).T
            wb[:, 514] = b1[e][0:128]
            wb[:, 515] = b1[e][128:256]
            bv = np.zeros((1, 8), np.float32)
            bv[0, 0] = b3_eff[e]
            in_maps.append(
                {
                    "f8": np.ascontiguousarray(f8),
                    "wb": wb.astype(bf16),
                    "brow": np.concatenate([b1[e], b2_eff[e]]).reshape(1, 512).astype(bf16),
                    "bv": bv,
                }
            )
        res = run_bass_kernel_spmd(nc, in_maps, core_ids=list(range(N_CORES)), **kwargs)

    LAST_EXEC_NS = res.exec_time_ns

    energies = np.zeros(Bsz, np.float64)
    for c in range(N_CORES):
        e, half = divmod(c, 2)
        ix = idxs[e]
        lo = half * S_used
        hi = min(len(ix), lo + S_used)
        if hi <= lo:
            continue
        evals = np.asarray(res.results[c]["out"]).reshape(-1)[: hi - lo]
        np.add.at(energies, ix[lo:hi] // Nn, evals.astype(np.float64))
    return energies.astype(np.float32)
